# revision 1
# baseline (speedup 1.0000x reference)
"""ChebyKAN layer on 8 Trainium2 NeuronCores (data-parallel over batch).

Computation:  out[b,o] = sum_{i,d} T_d(tanh(x)[b,i]) * C[i,o,d]
  - batch 32768 sharded 8 ways (4096 rows/core), coefficients replicated.
  - Per core: x-shard pre-transposed on host to [i=512, b=4096] so Chebyshev
    tiles sit [i partitions, b free]; PE contracts over (i,d) with cheby tiles
    as the stationary operand and C chunks [i,o] as the moving operand,
    accumulating out[b_tile=128, o=512] in PSUM over 33 chunks of 128.
  - d=0 (T_0 == 1) is folded: its four i-chunks are pre-summed on host into a
    single [128,512] chunk matmul'd against a constant ones tile.
"""

import os
from functools import lru_cache

import numpy as np
import ml_dtypes

import concourse.bass as bass
import concourse.mybir as mybir
import concourse.tile as tile
from concourse import bacc
from concourse.bass_utils import run_bass_kernel_spmd

N_CORES = 8
BATCH, IN_F, OUT_F, DEG = 32768, 512, 512, 8
B_LOC = BATCH // N_CORES  # 4096
P = 128
N_ICHUNK = IN_F // P  # 4
N_KCHUNK = DEG * N_ICHUNK  # 32 (d=0 handled as a bias add at copy-out)

MM_DT_NAME = os.environ.get("CHEBY_MM_DT", "f16")
_DT = {
    "bf16": (mybir.dt.bfloat16, ml_dtypes.bfloat16),
    "f16": (mybir.dt.float16, np.float16),
    "f32": (mybir.dt.float32, np.float32),
    "f32r": (mybir.dt.float32r, np.float32),
}
MM_DT, MM_NP = _DT[MM_DT_NAME]
# block of batch columns processed per iteration (SBUF-resident cheby tiles)
BBLK = 512 if MM_DT_NAME in ("bf16", "f16") else 256
# 1 = single K=128 matmul per chunk; 2 = two concurrent K=64 row-group tiles
KSPLIT = int(os.environ.get("CHEBY_KSPLIT", "1"))
# coefficients scaled up on host so fp16 C stays normal; undone at copy-out
C_SCALE = 1024.0 if MM_DT_NAME == "f16" else 1.0


def _build_kernel(reps=1):
    f32 = mybir.dt.float32
    nc = bacc.Bacc(
        "TRN2",
        target_bir_lowering=False,
        debug=False,
        num_devices=N_CORES,
    )
    xT = nc.declare_dram_parameter("xT", [IN_F, B_LOC], f32, isOutput=False)
    cw = nc.declare_dram_parameter("Cw", [N_KCHUNK * P, OUT_F], MM_DT, isOutput=False)
    bias = nc.declare_dram_parameter("bias", [1, OUT_F], f32, isOutput=False)
    out = nc.declare_dram_parameter("out", [B_LOC, OUT_F], f32, isOutput=True)

    xT_ap = xT[:, :].rearrange("(c p) b -> p c b", p=P)  # [128, 4, B_LOC]
    cw_ap = cw[:, :].rearrange("(k p) o -> p k o", p=P)  # [128, 32, 512]

    import contextlib

    with tile.TileContext(nc) as tc:
        with (
            tc.tile_pool(name="const", bufs=1) as const_pool,
            tc.tile_pool(name="xin", bufs=3) as xin_pool,
            tc.tile_pool(name="tf32", bufs=1) as f32_pool,
            tc.tile_pool(name="cheb", bufs=2) as cheb_pool,
            tc.tile_pool(name="ot", bufs=4) as out_pool,
            tc.tile_pool(name="ps", bufs=6 // KSPLIT, space="PSUM") as psum_pool,
        ):
            c_tile = const_pool.tile([P, N_KCHUNK, OUT_F], MM_DT)
            # split the C load so early k-chunks land before the first matmuls
            nsplit = 4
            per = (N_KCHUNK + nsplit - 1) // nsplit
            for s in range(nsplit):
                k0, k1 = s * per, min((s + 1) * per, N_KCHUNK)
                nc.gpsimd.dma_start(
                    out=c_tile[:, k0:k1, :], in_=cw_ap[:, k0:k1, :]
                )
            # bias row (the folded d=0 term) broadcast to all 128 partitions
            b_tile = const_pool.tile([P, OUT_F], f32)
            bias_ap = bias[:, :]
            bias_bcast = bass.AP(
                tensor=bias_ap.tensor,
                offset=bias_ap.offset,
                ap=[[0, P], bias_ap.ap[1]],
            )
            nc.gpsimd.dma_start(out=b_tile[:, :], in_=bias_bcast)

            rep_ctx = (
                tc.For_i(
                    0, reps, 1,
                    hint_engines=(
                        mybir.EngineType.PE,
                        mybir.EngineType.Activation,
                        mybir.EngineType.DVE,
                    ),
                )
                if reps > 1
                else contextlib.nullcontext()
            )
            with rep_ctx:
                _kernel_body(nc, tc, xT_ap, c_tile, b_tile, out,
                             xin_pool, f32_pool, cheb_pool, out_pool, psum_pool)
    nc.compile()
    return nc


def _kernel_body(nc, tc, xT_ap, c_tile, b_tile, out,
                 xin_pool, f32_pool, cheb_pool, out_pool, psum_pool):
    f32 = mybir.dt.float32
    MULT = mybir.AluOpType.mult
    ACT_F = mybir.ActivationFunctionType

    def stt(o, a, b):  # o = 2*a*b
        nc.vector.scalar_tensor_tensor(
            out=o, in0=a, scalar=2.0, in1=b, op0=MULT, op1=MULT
        )

    def sub1(o):  # o -= 1
        nc.vector.tensor_scalar(
            out=o, in0=o, scalar1=1.0, scalar2=None,
            op0=mybir.AluOpType.subtract,
        )

    for blk in range(B_LOC // BBLK):
        b0 = blk * BBLK
        x_in = xin_pool.tile([P, N_ICHUNK, BBLK], f32)
        nc.sync.dma_start(out=x_in[:, :, :], in_=xT_ap[:, :, b0 : b0 + BBLK])

        # Tf[:, j] = T_{j+1} in fp32 (j=0..3); Tb[:, j] = T_{j+1} in bf16 (j=0..7)
        Tf = f32_pool.tile([P, 4, N_ICHUNK, BBLK], f32)
        Tb = cheb_pool.tile([P, DEG, N_ICHUNK, BBLK], MM_DT)
        t1, t2, t3, t4 = (Tf[:, j, :, :] for j in range(4))
        nc.scalar.activation(out=t1, in_=x_in[:, :, :], func=ACT_F.Tanh)
        # fp32 chain: T2=2T1^2-1, T3=2T2T1-T1, T4=2T2^2-1
        stt(t2, t1, t1); sub1(t2)
        stt(t3, t2, t1); nc.vector.tensor_sub(t3, t3, t1)
        stt(t4, t2, t2); sub1(t4)
        # one-time rounding to bf16 on the scalar engine
        for j in range(4):
            nc.scalar.copy(out=Tb[:, j, :, :], in_=Tf[:, j, :, :])
        b1, b2, b3, b4 = (Tb[:, j, :, :] for j in range(4))
        b5, b6, b7, b8 = (Tb[:, j, :, :] for j in range(4, 8))
        # bf16 products: T5=2T3T2-T1, T6=2T3^2-1, T7=2T4T3-T1, T8=2T4^2-1
        stt(b5, b3, b2); nc.vector.tensor_sub(b5, b5, b1)
        stt(b6, b3, b3); sub1(b6)
        stt(b7, b4, b3); nc.vector.tensor_sub(b7, b7, b1)
        stt(b8, b4, b4); sub1(b8)

        for bt in range(BBLK // P):
            H = P // KSPLIT
            halves = [(h, h * H) for h in range(KSPLIT)]
            ps = [
                psum_pool.tile(
                    [P, OUT_F], f32, space="PSUM", tag=f"ps{h}", name=f"ps{h}"
                )
                for h in range(KSPLIT)
            ]
            bsl = slice(bt * P, (bt + 1) * P)
            for j in range(DEG):
                for c in range(N_ICHUNK):
                    k = j * N_ICHUNK + c
                    for h, lo in halves:
                        nc.tensor.matmul(
                            ps[h][:, :],
                            Tb[lo : lo + H, j, c, bsl],
                            c_tile[lo : lo + H, k, :],
                            start=(k == 0),
                            stop=(k == N_KCHUNK - 1),
                        )
            o_tile = out_pool.tile([P, OUT_F], f32)
            row = b0 + bt * P
            acc = ps[0][:, :]
            if KSPLIT > 1:
                half_sb = out_pool.tile([P, OUT_F], f32, tag="halfsb")
                nc.scalar.copy(out=half_sb[:, :], in_=ps[0][:, :])
                for h in range(1, KSPLIT - 1):
                    nc.vector.tensor_add(half_sb[:, :], half_sb[:, :], ps[h][:, :])
                nc.vector.tensor_add(half_sb[:, :], half_sb[:, :], ps[KSPLIT - 1][:, :])
                acc = half_sb[:, :]
            # out = psum / C_SCALE + bias   (bias = sum_i C[i,:,0], the d=0 term)
            nc.vector.scalar_tensor_tensor(
                out=o_tile[:, :],
                in0=acc,
                scalar=1.0 / C_SCALE,
                in1=b_tile[:, :],
                op0=MULT,
                op1=mybir.AluOpType.add,
            )
            nc.sync.dma_start(out=out[row : row + P, :], in_=o_tile[:, :])


@lru_cache(maxsize=4)
def _get_nc(reps=1):
    return _build_kernel(reps)


class Runner:
    """Persistent jitted runner mirroring bass2jax.run_bass_via_pjrt, reusable
    across calls (single jit cache entry) so repeated executions can be timed
    back-to-back without recompilation or host round-trips per call."""

    def __init__(self, nc):
        import jax
        import jax.numpy as jnp
        from jax.sharding import Mesh, PartitionSpec
        from jax.experimental.shard_map import shard_map
        from concourse import bass2jax
        from concourse import mybir as _mybir

        bass2jax.install_neuronx_cc_hook()
        self.jax = jax
        self.nc = nc
        partition_name = (
            nc.partition_id_tensor.name if nc.partition_id_tensor else None
        )
        in_names, out_names, out_avals = [], [], []
        for alloc in nc.m.functions[0].allocations:
            if not isinstance(alloc, _mybir.MemoryLocationSet):
                continue
            name = alloc.memorylocations[0].name
            if alloc.kind == "ExternalInput":
                if name != partition_name:
                    in_names.append(name)
            elif alloc.kind == "ExternalOutput":
                out_names.append(name)
                out_avals.append(
                    jax.core.ShapedArray(
                        tuple(alloc.tensor_shape), _mybir.dt.np(alloc.dtype)
                    )
                )
        self.in_names = list(in_names)
        self.out_names = out_names
        self.out_avals = out_avals
        n_params = len(in_names)
        all_names = in_names + out_names
        if partition_name is not None:
            all_names = all_names + [partition_name]

        def _body(*args):
            operands = list(args)
            if partition_name is not None:
                operands.append(bass2jax.partition_id_tensor())
            return tuple(
                bass2jax._bass_exec_p.bind(
                    *operands,
                    out_avals=tuple(out_avals),
                    in_names=tuple(all_names),
                    out_names=tuple(out_names),
                    lowering_input_output_aliases=(),
                    sim_require_finite=True,
                    sim_require_nnan=True,
                    nc=nc,
                )
            )

        devices = jax.devices()[:N_CORES]
        self.mesh = Mesh(np.asarray(devices), ("core",))
        in_specs = (PartitionSpec("core"),) * (n_params + len(out_names))
        out_specs = (PartitionSpec("core"),) * len(out_names)
        self.fn = jax.jit(
            shard_map(
                _body,
                mesh=self.mesh,
                in_specs=in_specs,
                out_specs=out_specs,
                check_rep=False,
            ),
            keep_unused=True,
        )

    def put_inputs(self, in_maps):
        import jax
        from jax.sharding import NamedSharding, PartitionSpec

        concat = [
            np.concatenate([np.asarray(m[name]) for m in in_maps], axis=0)
            for name in self.in_names
        ]
        for aval in self.out_avals:
            concat.append(
                np.zeros((N_CORES * aval.shape[0], *aval.shape[1:]), aval.dtype)
            )
        sh = NamedSharding(self.mesh, PartitionSpec("core"))
        return [jax.device_put(a, sh) for a in concat]

    def __call__(self, dev_inputs):
        return self.fn(*dev_inputs)

    def run_np(self, in_maps):
        outs = self(self.put_inputs(in_maps))
        return [
            {
                name: np.asarray(outs[i]).reshape(N_CORES, *self.out_avals[i].shape)[c]
                for i, name in enumerate(self.out_names)
            }
            for c in range(N_CORES)
        ]


def _prep_inputs(x: np.ndarray, coefficients: np.ndarray):
    x = np.asarray(x, dtype=np.float32)
    coefficients = np.asarray(coefficients, dtype=np.float32)
    # chunk k = j*4+c is degree j+1, i-chunk c, laid out [i within chunk, o];
    # the d=0 term (T_0 == 1) reduces to a bias row added at copy-out.
    c_perm = np.transpose(coefficients, (2, 0, 1))  # (d, i, o)
    bias = np.ascontiguousarray(c_perm[0].sum(axis=0, dtype=np.float64))
    bias = bias.astype(np.float32).reshape(1, OUT_F)
    c_main = c_perm[1:].reshape(N_KCHUNK * P, OUT_F) * C_SCALE
    c_all = np.ascontiguousarray(c_main).astype(MM_NP)

    in_maps = []
    for core in range(N_CORES):
        shard = x[core * B_LOC : (core + 1) * B_LOC]  # (4096, 512)
        xt = np.ascontiguousarray(shard.T)  # (512, 4096)
        in_maps.append({"xT": xt, "Cw": c_all, "bias": bias})
    return in_maps


@lru_cache(maxsize=4)
def _get_runner(reps=1):
    return Runner(_get_nc(reps))


def run_sharded(x, coefficients):
    """Run the 8-core kernel; returns the full (32768, 512) float32 output."""
    in_maps = _prep_inputs(x, coefficients)
    runner = _get_runner()
    results = runner.run_np(in_maps)
    parts = [np.asarray(results[i]["out"]) for i in range(N_CORES)]
    return np.concatenate(parts, axis=0).astype(np.float32)


def _time_runner(runner, dev_in, iters):
    import time

    outs = runner(dev_in)  # warm up
    outs[0].block_until_ready()
    times = []
    for _ in range(iters):
        t0 = time.perf_counter()
        outs = runner(dev_in)
        outs[0].block_until_ready()
        times.append((time.perf_counter() - t0) * 1e9)
    return times


def bench(x, coefficients, iters=12, rep_a=3, rep_b=83):
    """Estimate per-invocation HW time from the slope between two on-device
    repeat counts (fixed ~66-107ms axon RPC overhead cancels). Interleaved
    rounds + median to reject the bimodal RPC jitter. Returns
    (slope_ns, times_a, times_b)."""
    in_maps = _prep_inputs(x, coefficients)
    ra, rb = _get_runner(rep_a), _get_runner(rep_b)
    dev_a = ra.put_inputs(in_maps)
    dev_b = rb.put_inputs(in_maps)
    ta, tb = [], []
    for _ in range(3):
        ta += _time_runner(ra, dev_a, iters // 3 + 1)
        tb += _time_runner(rb, dev_b, iters // 3 + 1)
    med = lambda t: sorted(t)[len(t) // 2]
    slope = (med(tb) - med(ta)) / (rep_b - rep_a)
    return slope, ta, tb


def kernel(x, coefficients):
    return run_sharded(x, coefficients)



# revision 11
# speedup vs baseline: 1.2005x; 1.2005x over previous
"""ChebyKAN layer on 8 Trainium2 NeuronCores (data-parallel over batch).

Computation:  out[b,o] = sum_{i,d} T_d(tanh(x)[b,i]) * C[i,o,d]
  - batch 32768 sharded 8 ways (4096 rows/core), coefficients replicated.
  - Per core: x-shard pre-transposed on host to [i=512, b=4096]; Chebyshev
    tiles sit [i partitions, b free]; PE contracts over (i,d) with cheby
    tiles as the stationary operand and C chunks [i,o] as the moving
    operand, accumulating out[b_tile=128, o=512] in PSUM.

Chebyshev generation is spread across engines using shifted forms whose
constant offsets fold into a host-precomputed bias row:
    t1 = tanh(x)            [Act]
    T2h = t1^2              [DVE tt]     T2 = 2*T2h - 1   (C2 doubled, bias)
    t3  = (4*T2h-3)*t1      [DVE ts+tt]  true T3
    T4t = Sq(2rt2*T2h-rt2)  [Act]        T4 = T4t - 1     (bias fold)
    t5  = t3*(4*T2h-2) - t1 [DVE]        true T5
    T6h = Sq(t3)            [Act]        T6 = 2*T6h - 1   (C6 doubled, bias)
    t7  = (2*T4t-2)*t3 - t1 [DVE]        true T7
    T8t = Sq(rt2*T4t-rt2)   [Act]        T8 = T8t - 1     (bias fold)
DVE tensor_tensor/tensor_scalar fp16 run in 2x/4x modes; the old
scalar_tensor_tensor chain ran at 1x and left DVE ~90% busy.

Mode "mix8" additionally computes degrees 5 and 7 in fp8(e4m3) and issues
them as DoubleRow matmuls (two K=128 chunks per instruction, 2x MAC rate),
accumulated in a second PSUM bank with its own scale. Measured end-to-end
rel err ~1.6e-2 (gate 2e-2); mode "f16" stays at ~1.6e-3.
"""

import math
import os
from functools import lru_cache

import numpy as np
import ml_dtypes

import concourse.bass as bass
import concourse.mybir as mybir
import concourse.tile as tile
from concourse import bacc
from concourse.bass_utils import run_bass_kernel_spmd

N_CORES = 8
BATCH, IN_F, OUT_F, DEG = 32768, 512, 512, 8
B_LOC = BATCH // N_CORES  # 4096
P = 128
NIC = IN_F // P  # 4 i-chunks
BBLK = 512

MODE = os.environ.get("CHEBY_MODE", "mix8")  # "f16" | "mix8"
# degree -> (tile key, host-side C scale). Shifted tiles double C / fold bias.
DEGS_16 = [(1, "t1", 1.0), (2, "T2h", 2.0), (3, "t3", 1.0), (4, "T4t", 1.0),
           (6, "T6h", 2.0), (8, "T8t", 1.0)]
if MODE == "f16":
    DEGS_16 = DEGS_16 + [(5, "t5", 1.0), (7, "t7", 1.0)]
NK16 = len(DEGS_16) * NIC  # fp16 contraction chunks
# One scale for fp16 AND fp8 C so all matmuls share one PSUM accumulation
# group: x65536 keeps e4m3 C normal (max ~74 < 240) and is harmless in fp16.
CS = 65536.0
RT2 = math.sqrt(2.0)


def _build_kernel(reps=1):
    f32 = mybir.dt.float32
    f16 = mybir.dt.float16
    f8 = mybir.dt.float8e4
    nc = bacc.Bacc(
        "TRN2",
        target_bir_lowering=False,
        debug=False,
        num_devices=N_CORES,
    )
    xT = nc.declare_dram_parameter("xT", [IN_F, B_LOC], f32, isOutput=False)
    cw = nc.declare_dram_parameter("Cw", [NK16 * P, OUT_F], f16, isOutput=False)
    bias = nc.declare_dram_parameter("bias", [1, OUT_F], f32, isOutput=False)
    if MODE == "mix8":
        cw8 = nc.declare_dram_parameter("Cw8", [NIC * P, 2 * OUT_F], f8, isOutput=False)
    out = nc.declare_dram_parameter("out", [B_LOC, OUT_F], f32, isOutput=True)

    xT_ap = xT[:, :].rearrange("(c p) b -> p c b", p=P)  # [128, 4, B_LOC]
    cw_ap = cw[:, :].rearrange("(k p) o -> p k o", p=P)  # [128, NK16, 512]
    cw8_ap = (
        cw8[:, :].rearrange("(c p) (t o) -> p c t o", p=P, t=2)
        if MODE == "mix8" else None
    )

    import contextlib

    with tile.TileContext(nc) as tc:
        with (
            tc.tile_pool(name="const", bufs=1) as const_pool,
            tc.tile_pool(name="xin", bufs=3) as xin_pool,
            tc.tile_pool(name="tm", bufs=2) as tm_pool,
            tc.tile_pool(name="sc", bufs=2) as sc_pool,
            tc.tile_pool(name="ot", bufs=4) as out_pool,
            tc.tile_pool(name="ps", bufs=3, space="PSUM") as psum_pool,
        ):
            c_tile = const_pool.tile([P, NK16, OUT_F], f16)
            nsplit = 4
            per = (NK16 + nsplit - 1) // nsplit
            for s in range(nsplit):
                k0, k1 = s * per, min((s + 1) * per, NK16)
                nc.gpsimd.dma_start(out=c_tile[:, k0:k1, :], in_=cw_ap[:, k0:k1, :])
            if MODE == "mix8":
                c8_tile = const_pool.tile([P, NIC, 2, OUT_F], f8)
                nc.gpsimd.dma_start(out=c8_tile[:, :, :, :], in_=cw8_ap[:, :, :, :])
            else:
                c8_tile = None
            # per-partition scalar constant -sqrt(2) for activation bias
            nrt2 = const_pool.tile([P, 1], f32)
            nc.gpsimd.memset(nrt2[:, :], -RT2)
            # bias row broadcast to all 128 partitions
            b_tile = const_pool.tile([P, OUT_F], f32)
            bias_ap = bias[:, :]
            bias_bcast = bass.AP(
                tensor=bias_ap.tensor,
                offset=bias_ap.offset,
                ap=[[0, P], bias_ap.ap[1]],
            )
            nc.gpsimd.dma_start(out=b_tile[:, :], in_=bias_bcast)

            rep_ctx = (
                tc.For_i(
                    0, reps, 1,
                    hint_engines=(
                        mybir.EngineType.PE,
                        mybir.EngineType.Activation,
                        mybir.EngineType.DVE,
                    ),
                )
                if reps > 1
                else contextlib.nullcontext()
            )
            with rep_ctx:
                _kernel_body(nc, tc, xT_ap, c_tile, c8_tile, b_tile, nrt2, out,
                             xin_pool, tm_pool, sc_pool, out_pool, psum_pool)
    nc.compile()
    return nc


def _kernel_body(nc, tc, xT_ap, c_tile, c8_tile, b_tile, nrt2, out,
                 xin_pool, tm_pool, sc_pool, out_pool, psum_pool):
    f32 = mybir.dt.float32
    f16 = mybir.dt.float16
    f8 = mybir.dt.float8e4
    MULT = mybir.AluOpType.mult
    SUB = mybir.AluOpType.subtract
    ADD = mybir.AluOpType.add
    ACT_F = mybir.ActivationFunctionType

    for blk in range(B_LOC // BBLK):
        b0 = blk * BBLK
        x_in = xin_pool.tile([P, NIC, BBLK], f32)
        nc.sync.dma_start(out=x_in[:, :, :], in_=xT_ap[:, :, b0 : b0 + BBLK])

        t1 = tm_pool.tile([P, NIC, BBLK], f16, tag="t1")
        T2h = tm_pool.tile([P, NIC, BBLK], f16, tag="T2h")
        t3 = tm_pool.tile([P, NIC, BBLK], f16, tag="t3")
        T4t = tm_pool.tile([P, NIC, BBLK], f16, tag="T4t")
        T6h = tm_pool.tile([P, NIC, BBLK], f16, tag="T6h")
        T8t = tm_pool.tile([P, NIC, BBLK], f16, tag="T8t")
        v = sc_pool.tile([P, NIC, BBLK], f16, tag="v")
        t2d = sc_pool.tile([P, NIC, BBLK], f16, tag="t2d")
        t4d = sc_pool.tile([P, NIC, BBLK], f16, tag="t4d")
        m5 = sc_pool.tile([P, NIC, BBLK], f16, tag="m5")
        m7 = sc_pool.tile([P, NIC, BBLK], f16, tag="m7")
        if MODE == "mix8":
            T57 = tm_pool.tile([P, 2, NIC, BBLK], f8, tag="T57")
            t5_out, t7_out = T57[:, 0, :, :], T57[:, 1, :, :]
        else:
            t5 = tm_pool.tile([P, NIC, BBLK], f16, tag="t5")
            t7 = tm_pool.tile([P, NIC, BBLK], f16, tag="t7")
            t5_out, t7_out = t5[:, :, :], t7[:, :, :]

        A = lambda o, i, **kw: nc.scalar.activation(out=o, in_=i, **kw)
        tt = lambda o, a, b, op=MULT: nc.vector.tensor_tensor(out=o, in0=a, in1=b, op=op)
        ts = lambda o, i, s1, s2: nc.vector.tensor_scalar(
            out=o, in0=i, scalar1=s1, scalar2=s2, op0=MULT, op1=SUB)

        A(t1[:, :, :], x_in[:, :, :], func=ACT_F.Tanh)
        tt(T2h[:, :, :], t1[:, :, :], t1[:, :, :])
        ts(v[:, :, :], T2h[:, :, :], 4.0, 3.0)               # 2*T2-1
        tt(t3[:, :, :], v[:, :, :], t1[:, :, :])
        A(T4t[:, :, :], T2h[:, :, :], func=ACT_F.Square, scale=2 * RT2, bias=nrt2[:, :])
        A(T6h[:, :, :], t3[:, :, :], func=ACT_F.Square)
        A(T8t[:, :, :], T4t[:, :, :], func=ACT_F.Square, scale=RT2, bias=nrt2[:, :])
        ts(t2d[:, :, :], T2h[:, :, :], 4.0, 2.0)             # 2*T2
        tt(m5[:, :, :], t3[:, :, :], t2d[:, :, :])
        tt(t5_out, m5[:, :, :], t1[:, :, :], op=SUB)         # true T5
        ts(t4d[:, :, :], T4t[:, :, :], 2.0, 2.0)             # 2*T4
        tt(m7[:, :, :], t4d[:, :, :], t3[:, :, :])
        tt(t7_out, m7[:, :, :], t1[:, :, :], op=SUB)         # true T7

        tiles = {"t1": t1, "T2h": T2h, "t3": t3, "T4t": T4t, "T6h": T6h,
                 "T8t": T8t}
        if MODE == "f16":
            tiles["t5"], tiles["t7"] = t5, t7

        for bt in range(BBLK // P):
            bsl = slice(bt * P, (bt + 1) * P)
            ps1 = psum_pool.tile([P, OUT_F], f32, space="PSUM", tag="ps1", name="ps1")
            last_fp16 = MODE != "mix8"
            for k, (_, key, _s) in enumerate(DEGS_16):
                tl = tiles[key]
                for c in range(NIC):
                    kk = k * NIC + c
                    nc.tensor.matmul(
                        ps1[:, :],
                        tl[:, c, bsl],
                        c_tile[:, kk, :],
                        start=(kk == 0),
                        stop=(last_fp16 and kk == NK16 - 1),
                    )
            if MODE == "mix8":
                for c in range(NIC):
                    nc.tensor.matmul(
                        ps1[:, :],
                        T57[:, :, c, bsl],
                        c8_tile[:, c, :, :],
                        start=False,
                        stop=(c == NIC - 1),
                        perf_mode=mybir.MatmulPerfMode.DoubleRow,
                    )
            o_tile = out_pool.tile([P, OUT_F], f32)
            # out = ps / CS + bias
            nc.vector.scalar_tensor_tensor(
                out=o_tile[:, :], in0=ps1[:, :], scalar=1.0 / CS,
                in1=b_tile[:, :], op0=MULT, op1=ADD,
            )
            row = b0 + bt * P
            nc.sync.dma_start(out=out[row : row + P, :], in_=o_tile[:, :])


@lru_cache(maxsize=4)
def _get_nc(reps=1):
    return _build_kernel(reps)


class Runner:
    """Persistent jitted runner mirroring bass2jax.run_bass_via_pjrt, reusable
    across calls (single jit cache entry) so repeated executions can be timed
    back-to-back without recompilation or host round-trips per call."""

    def __init__(self, nc):
        import jax
        import jax.numpy as jnp
        from jax.sharding import Mesh, PartitionSpec
        from jax.experimental.shard_map import shard_map
        from concourse import bass2jax
        from concourse import mybir as _mybir

        bass2jax.install_neuronx_cc_hook()
        self.jax = jax
        self.nc = nc
        partition_name = (
            nc.partition_id_tensor.name if nc.partition_id_tensor else None
        )
        in_names, out_names, out_avals = [], [], []
        for alloc in nc.m.functions[0].allocations:
            if not isinstance(alloc, _mybir.MemoryLocationSet):
                continue
            name = alloc.memorylocations[0].name
            if alloc.kind == "ExternalInput":
                if name != partition_name:
                    in_names.append(name)
            elif alloc.kind == "ExternalOutput":
                out_names.append(name)
                out_avals.append(
                    jax.core.ShapedArray(
                        tuple(alloc.tensor_shape), _mybir.dt.np(alloc.dtype)
                    )
                )
        self.in_names = list(in_names)
        self.out_names = out_names
        self.out_avals = out_avals
        n_params = len(in_names)
        all_names = in_names + out_names
        if partition_name is not None:
            all_names = all_names + [partition_name]

        def _body(*args):
            operands = list(args)
            if partition_name is not None:
                operands.append(bass2jax.partition_id_tensor())
            return tuple(
                bass2jax._bass_exec_p.bind(
                    *operands,
                    out_avals=tuple(out_avals),
                    in_names=tuple(all_names),
                    out_names=tuple(out_names),
                    lowering_input_output_aliases=(),
                    sim_require_finite=True,
                    sim_require_nnan=True,
                    nc=nc,
                )
            )

        devices = jax.devices()[:N_CORES]
        self.mesh = Mesh(np.asarray(devices), ("core",))
        in_specs = (PartitionSpec("core"),) * (n_params + len(out_names))
        out_specs = (PartitionSpec("core"),) * len(out_names)
        self.fn = jax.jit(
            shard_map(
                _body,
                mesh=self.mesh,
                in_specs=in_specs,
                out_specs=out_specs,
                check_rep=False,
            ),
            keep_unused=True,
        )

    def put_inputs(self, in_maps):
        import jax
        from jax.sharding import NamedSharding, PartitionSpec

        concat = [
            np.concatenate([np.asarray(m[name]) for m in in_maps], axis=0)
            for name in self.in_names
        ]
        for aval in self.out_avals:
            concat.append(
                np.zeros((N_CORES * aval.shape[0], *aval.shape[1:]), aval.dtype)
            )
        sh = NamedSharding(self.mesh, PartitionSpec("core"))
        return [jax.device_put(a, sh) for a in concat]

    def __call__(self, dev_inputs):
        return self.fn(*dev_inputs)

    def run_np(self, in_maps):
        outs = self(self.put_inputs(in_maps))
        return [
            {
                name: np.asarray(outs[i]).reshape(N_CORES, *self.out_avals[i].shape)[c]
                for i, name in enumerate(self.out_names)
            }
            for c in range(N_CORES)
        ]


def _prep_inputs(x: np.ndarray, coefficients: np.ndarray):
    x = np.asarray(x, dtype=np.float32)
    C = np.asarray(coefficients, dtype=np.float64)  # (in, out, deg+1)

    # bias row: d=0 plus the folded constants of the shifted tiles
    bias = (C[:, :, 0] - C[:, :, 2] - C[:, :, 4] - C[:, :, 6] - C[:, :, 8]).sum(axis=0)
    bias = bias.astype(np.float32).reshape(1, OUT_F)

    # fp16 chunks: k = deg_idx*NIC + ic, rows [i within chunk], cols o
    cw = np.empty((NK16 * P, OUT_F), np.float32)
    for k, (d, _key, scale) in enumerate(DEGS_16):
        for c in range(NIC):
            cw[(k * NIC + c) * P : (k * NIC + c + 1) * P] = (
                C[c * P : (c + 1) * P, :, d] * (scale * CS)
            )
    c_all = cw.astype(np.float16)

    in_extra = {}
    if MODE == "mix8":
        c8 = np.empty((NIC, P, 2, OUT_F), np.float32)
        for c in range(NIC):
            c8[c, :, 0, :] = C[c * P : (c + 1) * P, :, 5] * CS
            c8[c, :, 1, :] = C[c * P : (c + 1) * P, :, 7] * CS
        in_extra["Cw8"] = c8.reshape(NIC * P, 2 * OUT_F).astype(ml_dtypes.float8_e4m3)

    in_maps = []
    for core in range(N_CORES):
        shard = x[core * B_LOC : (core + 1) * B_LOC]  # (4096, 512)
        xt = np.ascontiguousarray(shard.T)  # (512, 4096)
        m = {"xT": xt, "Cw": c_all, "bias": bias}
        m.update(in_extra)
        in_maps.append(m)
    return in_maps


@lru_cache(maxsize=4)
def _get_runner(reps=1):
    return Runner(_get_nc(reps))


def run_sharded(x, coefficients):
    """Run the 8-core kernel; returns the full (32768, 512) float32 output."""
    in_maps = _prep_inputs(x, coefficients)
    runner = _get_runner()
    results = runner.run_np(in_maps)
    parts = [np.asarray(results[i]["out"]) for i in range(N_CORES)]
    return np.concatenate(parts, axis=0).astype(np.float32)


def _time_runner(runner, dev_in, iters):
    import time

    outs = runner(dev_in)  # warm up
    outs[0].block_until_ready()
    times = []
    for _ in range(iters):
        t0 = time.perf_counter()
        outs = runner(dev_in)
        outs[0].block_until_ready()
        times.append((time.perf_counter() - t0) * 1e9)
    return times


def bench(x, coefficients, iters=12, rep_a=3, rep_b=83):
    """Estimate per-invocation HW time from the slope between two on-device
    repeat counts (fixed ~66-107ms axon RPC overhead cancels). Interleaved
    rounds + median to reject the bimodal RPC jitter. Returns
    (slope_ns, times_a, times_b)."""
    in_maps = _prep_inputs(x, coefficients)
    ra, rb = _get_runner(rep_a), _get_runner(rep_b)
    dev_a = ra.put_inputs(in_maps)
    dev_b = rb.put_inputs(in_maps)
    ta, tb = [], []
    for _ in range(3):
        ta += _time_runner(ra, dev_a, iters // 3 + 1)
        tb += _time_runner(rb, dev_b, iters // 3 + 1)
    med = lambda t: sorted(t)[len(t) // 2]
    slope = (med(tb) - med(ta)) / (rep_b - rep_a)
    return slope, ta, tb


def kernel(x, coefficients):
    return run_sharded(x, coefficients)
